# revision 15
# baseline (speedup 1.0000x reference)
"""AttnBlock (GroupNorm -> q/k/v 1x1 conv -> single-head attention -> proj
-> residual) on 8 Trainium2 NeuronCores.

Sharding: core i handles batch b = i//2, token half t = i%2. Each core's
x image is host-rolled along the token dim so its 2048 query tokens are
always local columns 0..2047 -- attention is permutation-invariant over
keys and GroupNorm over space, so all 8 cores run one SPMD program.
Each core redundantly computes GN + k/v for its full image (cheap) and
q/attention/proj for its half.

Device layout: channels on partitions in 4 tiles of 128. scores^T is
computed key-major (lhsT=k, rhs=q) so softmax-exp tiles feed the AV
matmul directly with no transposes; the softmax sum Z comes from a
ones-vector matmul and 1/Z is folded after AV (v-bias folds exactly via
sum(attn)==1). GroupNorm cross-partition combine and the 1/Z partition
broadcast run on tiny DMAs, keeping every matmul's wait set small
(walrus rejects matmuls with too many sync waits).
"""

import os
import sys

import numpy as np

for _p in ("/opt/trn_rl_repo", "/root/.axon_site/_ro/trn_rl_repo"):
    if os.path.isdir(_p) and _p not in sys.path:
        sys.path.insert(0, _p)

os.environ.setdefault("MYCRO_LOCAL_CACHE", "1")

import ml_dtypes  # noqa: E402

import concourse.bacc as bacc  # noqa: E402
import concourse.bass as bass  # noqa: E402
import concourse.mybir as mybir  # noqa: E402
import concourse.tile as tile  # noqa: E402
from concourse.bass_utils import run_bass_kernel_spmd  # noqa: E402

F32 = mybir.dt.float32
BF16 = mybir.dt.bfloat16
AF = mybir.ActivationFunctionType
OP = mybir.AluOpType

B = 4
C = 512
HW = 4096
NH = HW // 2  # tokens per core
CT = C // 128  # channel tiles
NB = 512  # token block for matmul free dim
NBLK = NH // NB
MCH = HW // 128  # key chunks of 128
NG = 8  # groups per channel tile (128/16)
GROUP = 16
EPS = 1e-6
SCL = 1.0 / float(np.sqrt(C))
N_CORES = 8
W_NAMES = ("wqT", "wkT", "wvT", "wpT")
V_NAMES = ("bq", "bk", "bv", "bp", "gamma", "beta")

_NC = None


def _rep_ap(src, ap):
    """Manual access pattern on a tile: list of [step, count] with the
    partition dim first (kept from src)."""
    return bass.AP(tensor=src.tensor, offset=src.offset, ap=ap)


def _emit(nc, tc, t):
    from contextlib import ExitStack

    with ExitStack() as es:
        const = es.enter_context(tc.tile_pool(name="const", bufs=1))
        big = es.enter_context(tc.tile_pool(name="big", bufs=1))
        ps = es.enter_context(tc.tile_pool(name="ps", bufs=1, space="PSUM"))

        w_sb = const.tile([128, len(W_NAMES), CT, C], BF16, tag="w")
        nc.sync.dma_start(out=w_sb, in_=t["wAll"][:, :, :].rearrange(
            "w (t p) o -> p w t o", p=128))
        vec_sb = const.tile([128, len(V_NAMES), CT], F32, tag="vecs")
        nc.sync.dma_start(out=vec_sb, in_=t["vecs"][:, :].rearrange(
            "v (t p) -> p v t", p=128))
        WQ, WK, WV, WP = range(4)
        BQ, BK, BV, BP, GAMMA, BETA = range(6)
        ones_bf = const.tile([128, 1], BF16, tag="ones_bf")
        nc.vector.memset(ones_bf, 1.0)
        eps_sb = const.tile([128, 1], F32, tag="eps")
        nc.vector.memset(eps_sb, EPS)

        x = t["x_img"]
        y = t["y"]

        # ---------- Phase A: GroupNorm -> h (bf16, [128, CT, HW]) ----------
        with tc.tile_pool(name="hp", bufs=1) as hp:
            h_sb = hp.tile([128, CT, HW], BF16, tag="h")
            with tc.tile_pool(name="xp", bufs=2) as xp, \
                    tc.tile_pool(name="gn", bufs=3) as gn:
                fmax = nc.vector.BN_STATS_FMAX
                nsub = HW // fmax
                for tt in range(CT):
                    xt = xp.tile([128, HW], F32, tag="xt")
                    nc.sync.dma_start(out=xt, in_=x[tt * 128:(tt + 1) * 128, :])
                    xr = xt.rearrange("p (s f) -> p s f", f=fmax)
                    st = gn.tile([128, nsub, nc.vector.BN_STATS_DIM], F32, tag="st")
                    for si in range(nsub):
                        nc.vector.bn_stats(out=st[:, si, :], in_=xr[:, si, :])
                    mv = gn.tile([128, 2], F32, tag="mv")
                    nc.vector.bn_aggr(out=mv, in_=st)
                    # S = [mean, E[x^2]] per partition
                    S = gn.tile([128, 2], F32, tag="S")
                    nc.vector.tensor_copy(out=S[:, 0:1], in_=mv[:, 0:1])
                    nc.vector.scalar_tensor_tensor(
                        out=S[:, 1:2], in0=mv[:, 0:1], scalar=mv[:, 0:1],
                        in1=mv[:, 1:2], op0=OP.mult, op1=OP.add)
                    # gather all 128 partitions' stats onto one partition
                    Sg = gn.tile([1, 128 * 2], F32, tag="Sg")
                    nc.sync.dma_start(
                        out=Sg.rearrange("o (p c) -> o p c", c=2), in_=S)
                    # group-reduce: flat f = g*32 + s*2 + c -> view [g, c, s]
                    Gs = gn.tile([1, NG, 2], F32, tag="Gs")
                    nc.vector.reduce_sum(
                        out=Gs,
                        in_=Sg.rearrange("o (g s c) -> o g c s", g=NG, c=2),
                        axis=mybir.AxisListType.X)
                    nc.vector.tensor_scalar_mul(
                        out=Gs.rearrange("o g c -> o (g c)"),
                        in0=Gs.rearrange("o g c -> o (g c)"),
                        scalar1=1.0 / GROUP)
                    mu2 = gn.tile([1, NG], F32, tag="mu2")
                    nc.vector.tensor_mul(out=mu2, in0=Gs[:, :, 0], in1=Gs[:, :, 0])
                    var = gn.tile([1, NG], F32, tag="var")
                    nc.vector.tensor_sub(out=var, in0=Gs[:, :, 1], in1=mu2)
                    sd = gn.tile([1, NG], F32, tag="sd")
                    nc.scalar.activation(out=sd, in_=var, func=AF.Sqrt,
                                         bias=eps_sb[0:1, :])
                    rstd = gn.tile([1, NG], F32, tag="rstd")
                    nc.vector.reciprocal(out=rstd, in_=sd)
                    # broadcast per-group mu and rstd back to per-partition:
                    # expand each [1, NG] -> [1, 128] (repeat 16x) on DVE, then
                    # cross partitions with two small DMAs
                    pk = gn.tile([1, 256], F32, tag="pk")
                    nc.vector.tensor_copy(
                        out=pk[:, 0:128],
                        in_=_rep_ap(Gs, [Gs.ap[0], [2, NG], [0, GROUP]]))
                    nc.vector.tensor_copy(
                        out=pk[:, 128:256],
                        in_=_rep_ap(rstd, [rstd.ap[0], [1, NG], [0, GROUP]]))
                    gmu = gn.tile([128, 1], F32, tag="gmu")
                    nc.sync.dma_start(out=gmu, in_=pk[:, 0:128])
                    grs = gn.tile([128, 1], F32, tag="grs")
                    nc.sync.dma_start(out=grs, in_=pk[:, 128:256])
                    A = gn.tile([128, 1], F32, tag="A")
                    nc.vector.tensor_mul(out=A, in0=grs,
                                         in1=vec_sb[:, GAMMA, tt:tt + 1])
                    muA = gn.tile([128, 1], F32, tag="muA")
                    nc.vector.tensor_mul(out=muA, in0=gmu, in1=A)
                    Bb = gn.tile([128, 1], F32, tag="Bb")
                    nc.vector.tensor_sub(out=Bb, in0=vec_sb[:, BETA, tt:tt + 1],
                                         in1=muA)
                    # h = x*A + Bb, cast to bf16
                    nc.scalar.activation(out=h_sb[:, tt, :], in_=xt,
                                         func=AF.Identity, bias=Bb, scale=A)

            # ---------- Phase B: q/k/vT 1x1 convs ----------
            k_sb = big.tile([128, CT, HW], BF16, tag="k")
            vT_sb = big.tile([128, MCH, C], BF16, tag="vT")
            q_sb = big.tile([128, CT, NH], BF16, tag="q")
            # k over all HW tokens: [o-part, m]
            for oo in range(CT):
                for mb in range(HW // NB):
                    pp = ps.tile([128, NB], F32, tag="sp", bufs=3)
                    for kk in range(CT):
                        nc.tensor.matmul(
                            pp, w_sb[:, WK, kk, oo * 128:(oo + 1) * 128],
                            h_sb[:, kk, mb * NB:(mb + 1) * NB],
                            start=(kk == 0), stop=(kk == CT - 1))
                    nc.scalar.activation(out=k_sb[:, oo, mb * NB:(mb + 1) * NB],
                                         in_=pp, func=AF.Identity,
                                         bias=vec_sb[:, BK, oo:oo + 1])
            # q over my NH tokens
            for oo in range(CT):
                for nb in range(NBLK):
                    pp = ps.tile([128, NB], F32, tag="sp", bufs=3)
                    for kk in range(CT):
                        nc.tensor.matmul(
                            pp, w_sb[:, WQ, kk, oo * 128:(oo + 1) * 128],
                            h_sb[:, kk, nb * NB:(nb + 1) * NB],
                            start=(kk == 0), stop=(kk == CT - 1))
                    nc.scalar.activation(out=q_sb[:, oo, nb * NB:(nb + 1) * NB],
                                         in_=pp, func=AF.Identity,
                                         bias=vec_sb[:, BQ, oo:oo + 1])
            # vT token-major: [m-part, o]; v bias folded in after AV
            for j in range(MCH):
                pp = ps.tile([128, C], F32, tag="sp", bufs=3)
                for kk in range(CT):
                    nc.tensor.matmul(
                        pp, h_sb[:, kk, j * 128:(j + 1) * 128],
                        w_sb[:, WV, kk, :],
                        start=(kk == 0), stop=(kk == CT - 1))
                nc.scalar.copy(out=vT_sb[:, j, :], in_=pp)

        # ---------- Phase C: attention + proj + residual, per n-block ----------
        with tc.tile_pool(name="expp", bufs=2) as expp, \
                tc.tile_pool(name="attp", bufs=1) as attp, \
                tc.tile_pool(name="outp", bufs=3) as outp:
            proj_pending = None

            def proj_block(nb, ao):
                n0 = nb * NB
                for oo in range(CT):
                    pp = ps.tile([128, NB], F32, tag="sp", bufs=3)
                    for cc in range(CT):
                        nc.tensor.matmul(
                            pp, w_sb[:, WP, cc, oo * 128:(oo + 1) * 128],
                            ao[:, cc, :],
                            start=(cc == 0), stop=(cc == CT - 1))
                    yf = outp.tile([128, NB], F32, tag="yf")
                    nc.scalar.activation(out=yf, in_=pp, func=AF.Identity,
                                         bias=vec_sb[:, BP, oo:oo + 1])
                    nc.gpsimd.dma_start(
                        out=yf, in_=x[oo * 128:(oo + 1) * 128, n0:n0 + NB],
                        accum_op=OP.add)
                    nc.sync.dma_start(
                        out=y[oo * 128:(oo + 1) * 128, n0:n0 + NB], in_=yf)

            for nb in range(NBLK):
                n0 = nb * NB
                ex = expp.tile([128, MCH, NB], BF16, tag="ex")
                zps = ps.tile([1, NB], F32, tag="z")
                # scores^T chunk j: [m 128, n NB] = k_chunk^T q_block; exp on
                # ACT. Z accumulation lags one chunk so PE never waits on ACT.
                for j in range(MCH):
                    sp = ps.tile([128, NB], F32, tag="sp", bufs=3)
                    for kk in range(CT):
                        nc.tensor.matmul(
                            sp, k_sb[:, kk, j * 128:(j + 1) * 128],
                            q_sb[:, kk, n0:n0 + NB],
                            start=(kk == 0), stop=(kk == CT - 1))
                    nc.scalar.activation(out=ex[:, j, :], in_=sp, func=AF.Exp,
                                         scale=SCL)
                    if j >= 1:
                        nc.tensor.matmul(zps, ones_bf, ex[:, j - 1, :],
                                         start=(j == 1), stop=False)
                # previous n-block's proj runs here on PE while this block's
                # softmax-normalize chain (ACT/DVE/DMA) completes
                if proj_pending is not None:
                    proj_block(*proj_pending)
                # U[c, n] = sum_m v[c, m] exp^T[m, n]
                U = ps.tile([128, CT, NB], F32, tag="U")
                for j in range(MCH):
                    for cc in range(CT):
                        nc.tensor.matmul(
                            U[:, cc, :], vT_sb[:, j, cc * 128:(cc + 1) * 128],
                            ex[:, j, :],
                            start=(j == 0), stop=(j == MCH - 1))
                nc.tensor.matmul(zps, ones_bf, ex[:, MCH - 1, :],
                                 start=False, stop=True)
                zsb = attp.tile([1, NB], F32, tag="zsb")
                nc.scalar.copy(out=zsb, in_=zps)
                rz = attp.tile([1, NB], F32, tag="rz")
                nc.vector.reciprocal(out=rz, in_=zsb)
                rzb = attp.tile([128, NB], F32, tag="rzb")
                nc.sync.dma_start(
                    out=rzb, in_=_rep_ap(rz, [rz.ap[0], [0, 128], [1, NB]]))
                Usb = attp.tile([128, CT, NB], F32, tag="Usb")
                for cc in range(CT):
                    nc.scalar.copy(out=Usb[:, cc, :], in_=U[:, cc, :])
                ao = attp.tile([128, CT, NB], BF16, tag="ao")
                for cc in range(CT):
                    un = attp.tile([128, NB], F32, tag="un")
                    nc.vector.tensor_mul(out=un, in0=Usb[:, cc, :], in1=rzb)
                    nc.vector.tensor_scalar_add(out=ao[:, cc, :], in0=un,
                                                scalar1=vec_sb[:, BV, cc:cc + 1])
                proj_pending = (nb, ao)
            proj_block(*proj_pending)


def _build_program():
    nc = bacc.Bacc()
    t = {}
    t["x_img"] = nc.dram_tensor("x_img", [C, HW], F32, kind="ExternalInput")
    t["wAll"] = nc.dram_tensor("wAll", [len(W_NAMES), C, C], BF16,
                               kind="ExternalInput")
    t["vecs"] = nc.dram_tensor("vecs", [len(V_NAMES), C], F32,
                               kind="ExternalInput")
    t["y"] = nc.dram_tensor("y", [C, NH], F32, kind="ExternalOutput")
    with tile.TileContext(nc) as tc:
        _emit(nc, tc, t)
    nc.compile()
    return nc


def _get_program():
    global _NC
    if _NC is None:
        _NC = _build_program()
    return _NC


def _make_in_maps(inputs):
    f32 = np.float32
    bf16 = ml_dtypes.bfloat16
    xs = np.asarray(inputs["x"], f32).reshape(B, C, HW)
    wAll = np.stack([np.asarray(inputs[k], f32).T
                     for k in ("Wq", "Wk", "Wv", "Wp")]).astype(bf16)
    vecs = np.stack([np.asarray(inputs[k], f32)
                     for k in ("bq", "bk", "bv", "bp", "gamma", "beta")])
    shared = {"wAll": np.ascontiguousarray(wAll),
              "vecs": np.ascontiguousarray(vecs)}
    in_maps = []
    for core in range(N_CORES):
        b, t = core // 2, core % 2
        xi = xs[b]
        if t:
            xi = np.roll(xi, -NH, axis=1)
        in_maps.append({"x_img": np.ascontiguousarray(xi), **shared})
    return in_maps


def _assemble(results):
    out = np.empty((B, C, HW), np.float32)
    for core in range(N_CORES):
        b, t = core // 2, core % 2
        out[b][:, t * NH:(t + 1) * NH] = results[core]["y"]
    return out.reshape(B, C, HW // 64, 64)


def _run(inputs, **kwargs):
    nc = _get_program()
    in_maps = _make_in_maps(inputs)
    bkr = run_bass_kernel_spmd(nc, in_maps, list(range(N_CORES)), **kwargs)
    return _assemble(bkr.results), bkr


def kernel(**inputs):
    out, _ = _run(inputs)
    return out


# revision 26
# speedup vs baseline: 4.5418x; 4.5418x over previous
"""AttnBlock (GroupNorm -> q/k/v 1x1 conv -> single-head attention -> proj
-> residual) on 8 Trainium2 NeuronCores.

Sharding: core i handles batch b = i//2, token half t = i%2. Each core's
x image is host-rolled along the token dim so its 2048 query tokens are
always local columns 0..2047 -- attention is permutation-invariant over
keys and GroupNorm over space, so all 8 cores run one SPMD program.
Each core redundantly computes GN + k/v for its full image (cheap) and
q/attention/proj for its half.

Device layout: channels on partitions in 4 tiles of 128. scores^T is
computed key-major (lhsT=k, rhs=q) so softmax-exp tiles feed the AV
matmul directly with no transposes; the softmax sum Z comes from a
ones-vector matmul and 1/Z is folded after AV (v-bias folds exactly via
sum(attn)==1). GroupNorm cross-partition combine and the 1/Z partition
broadcast run on tiny DMAs, keeping every matmul's wait set small
(walrus rejects matmuls with too many sync waits).
"""

import os
import sys

import numpy as np

for _p in ("/opt/trn_rl_repo", "/root/.axon_site/_ro/trn_rl_repo"):
    if os.path.isdir(_p) and _p not in sys.path:
        sys.path.insert(0, _p)

os.environ.setdefault("MYCRO_LOCAL_CACHE", "1")

import ml_dtypes  # noqa: E402

import concourse.bacc as bacc  # noqa: E402
import concourse.bass as bass  # noqa: E402
import concourse.mybir as mybir  # noqa: E402
import concourse.tile as tile  # noqa: E402
from concourse.bass_utils import run_bass_kernel_spmd  # noqa: E402

F32 = mybir.dt.float32
BF16 = mybir.dt.bfloat16
AF = mybir.ActivationFunctionType
OP = mybir.AluOpType

B = 4
C = 512
HW = 4096
NH = HW // 2  # tokens per core
CT = C // 128  # channel tiles
NB = 512  # token block for matmul free dim
NBLK = NH // NB
MCH = HW // 128  # key chunks of 128
NG = 8  # groups per channel tile (128/16)
GROUP = 16
EPS = 1e-6
SCL = 1.0 / float(np.sqrt(C))
N_CORES = 8
W_NAMES = ("wqT", "wkT", "wvT", "wpT")
V_NAMES = ("bq", "bk", "bv", "bp", "gamma", "beta")

_NC = None


def _rep_ap(src, ap):
    """Manual access pattern on a tile: list of [step, count] with the
    partition dim first (kept from src)."""
    return bass.AP(tensor=src.tensor, offset=src.offset, ap=ap)


def _emit(nc, tc, t):
    from contextlib import ExitStack

    with ExitStack() as es:
        const = es.enter_context(tc.tile_pool(name="const", bufs=1))
        big = es.enter_context(tc.tile_pool(name="big", bufs=1))
        ps = es.enter_context(tc.tile_pool(name="ps", bufs=1, space="PSUM"))

        vec_sb = const.tile([128, len(V_NAMES), CT], F32, tag="vecs")
        nc.sync.dma_start(out=vec_sb, in_=t["vecs"][:, :].rearrange(
            "v (t p) -> p v t", p=128))
        WQ, WK, WV, WP = range(4)
        BQ, BK, BV, BP, GAMMA, BETA = range(6)
        ones_f32 = const.tile([128, 1], F32, tag="ones_f32")
        nc.vector.memset(ones_f32, 1.0)
        eps_sb = const.tile([128, 1], F32, tag="eps")
        nc.vector.memset(eps_sb, EPS)
        gmap_sb = const.tile([128, NG], F32, tag="gmap")
        nc.sync.dma_start(out=gmap_sb, in_=t["gmap"][:, :])
        gmapT_sb = const.tile([NG, 128], F32, tag="gmapT")
        nc.sync.dma_start(out=gmapT_sb, in_=t["gmapT"][:, :])

        x = t["x_img"]
        y = t["y"]

        # ---------- Phase A: GroupNorm -> h (bf16, [128, CT, HW]) ----------
        with tc.tile_pool(name="hp", bufs=1) as hp:
            h_sb = hp.tile([128, CT, HW], BF16, tag="h")
            with tc.tile_pool(name="xp", bufs=2) as xp, \
                    tc.tile_pool(name="gn", bufs=3) as gn:
                fmax = nc.vector.BN_STATS_FMAX
                nsub = HW // fmax
                for tt in range(CT):
                    xt = xp.tile([128, HW], BF16, tag="xt")
                    # two chunks so bn_stats starts before the full row lands
                    nc.sync.dma_start(
                        out=xt[:, 0:HW // 2],
                        in_=t["xh"][tt * 128:(tt + 1) * 128, 0:HW // 2])
                    nc.sync.dma_start(
                        out=xt[:, HW // 2:HW],
                        in_=t["xh"][tt * 128:(tt + 1) * 128, HW // 2:HW])
                    xr = xt.rearrange("p (s f) -> p s f", f=fmax)
                    st = gn.tile([128, nsub, nc.vector.BN_STATS_DIM], F32, tag="st")
                    for si in range(nsub):
                        nc.vector.bn_stats(out=st[:, si, :], in_=xr[:, si, :])
                    mv = gn.tile([128, 2], F32, tag="mv")
                    nc.vector.bn_aggr(out=mv, in_=st)
                    # S = [mean, E[x^2]] per partition
                    S = gn.tile([128, 2], F32, tag="S")
                    nc.vector.tensor_copy(out=S[:, 0:1], in_=mv[:, 0:1])
                    nc.vector.scalar_tensor_tensor(
                        out=S[:, 1:2], in0=mv[:, 0:1], scalar=mv[:, 0:1],
                        in1=mv[:, 1:2], op0=OP.mult, op1=OP.add)
                    # combine the 16 partitions of each group (8 groups/tile)
                    # via tiny PE matmuls: gmap sums+scales, gmapT broadcasts
                    gps = ps.tile([NG, 2], F32, tag="U")
                    nc.tensor.matmul(gps, gmap_sb, S, start=True, stop=True)
                    gsb = gn.tile([NG, 2], F32, tag="gsb")
                    nc.vector.tensor_copy(out=gsb, in_=gps)
                    bps = ps.tile([128, 2], F32, tag="z")
                    nc.tensor.matmul(bps, gmapT_sb, gsb, start=True, stop=True)
                    gstat = gn.tile([128, 2], F32, tag="gstat")
                    nc.vector.tensor_copy(out=gstat, in_=bps)
                    mu = gstat[:, 0:1]
                    mu2 = gn.tile([128, 1], F32, tag="mu2")
                    nc.vector.tensor_mul(out=mu2, in0=mu, in1=mu)
                    var = gn.tile([128, 1], F32, tag="var")
                    nc.vector.tensor_sub(out=var, in0=gstat[:, 1:2], in1=mu2)
                    sd = gn.tile([128, 1], F32, tag="sd")
                    nc.scalar.activation(out=sd, in_=var, func=AF.Sqrt,
                                         bias=eps_sb)
                    rstd = gn.tile([128, 1], F32, tag="rstd")
                    nc.vector.reciprocal(out=rstd, in_=sd)
                    A = gn.tile([128, 1], F32, tag="A")
                    nc.vector.tensor_mul(out=A, in0=rstd,
                                         in1=vec_sb[:, GAMMA, tt:tt + 1])
                    muA = gn.tile([128, 1], F32, tag="muA")
                    nc.vector.tensor_mul(out=muA, in0=mu, in1=A)
                    Bb = gn.tile([128, 1], F32, tag="Bb")
                    nc.vector.tensor_sub(out=Bb, in0=vec_sb[:, BETA, tt:tt + 1],
                                         in1=muA)
                    # h = x*A + Bb (bf16 in/out)
                    nc.scalar.activation(out=h_sb[:, tt, :], in_=xt,
                                         func=AF.Identity, bias=Bb, scale=A)

            # weights loaded after the x chunks so GN starts ASAP
            w_sb = const.tile([128, len(W_NAMES), CT, C], BF16, tag="w")
            nc.sync.dma_start(out=w_sb, in_=t["wAll"][:, :, :].rearrange(
                "w (t p) o -> p w t o", p=128))

            # ---------- Phase B: q/k/vT 1x1 convs ----------
            k_sb = big.tile([128, CT, HW], BF16, tag="k")
            vT_sb = big.tile([128, MCH, C], BF16, tag="vT")
            q_sb = big.tile([128, CT, NH], BF16, tag="q")
            # k over all HW tokens: [o-part, m]
            for oo in range(CT):
                for mb in range(HW // NB):
                    pp = ps.tile([128, NB], F32, tag="sp", bufs=3)
                    for kk in range(CT):
                        nc.tensor.matmul(
                            pp, w_sb[:, WK, kk, oo * 128:(oo + 1) * 128],
                            h_sb[:, kk, mb * NB:(mb + 1) * NB],
                            start=(kk == 0), stop=(kk == CT - 1))
                    nc.scalar.activation(out=k_sb[:, oo, mb * NB:(mb + 1) * NB],
                                         in_=pp, func=AF.Identity,
                                         bias=vec_sb[:, BK, oo:oo + 1])
            # q over my NH tokens
            for oo in range(CT):
                for nb in range(NBLK):
                    pp = ps.tile([128, NB], F32, tag="sp", bufs=3)
                    for kk in range(CT):
                        nc.tensor.matmul(
                            pp, w_sb[:, WQ, kk, oo * 128:(oo + 1) * 128],
                            h_sb[:, kk, nb * NB:(nb + 1) * NB],
                            start=(kk == 0), stop=(kk == CT - 1))
                    nc.scalar.activation(out=q_sb[:, oo, nb * NB:(nb + 1) * NB],
                                         in_=pp, func=AF.Identity,
                                         bias=vec_sb[:, BQ, oo:oo + 1])
            # vT token-major: [m-part, o]; v bias folded in after AV
            for j in range(MCH):
                pp = ps.tile([128, C], F32, tag="sp", bufs=3)
                for kk in range(CT):
                    nc.tensor.matmul(
                        pp, h_sb[:, kk, j * 128:(j + 1) * 128],
                        w_sb[:, WV, kk, :],
                        start=(kk == 0), stop=(kk == CT - 1))
                nc.scalar.copy(out=vT_sb[:, j, :], in_=pp)

        # ---------- Phase C: attention + proj + residual, per n-block ----------
        with tc.tile_pool(name="expp", bufs=2) as expp, \
                tc.tile_pool(name="attp", bufs=1) as attp, \
                tc.tile_pool(name="outp", bufs=3) as outp:
            proj_pending = None

            def proj_block(nb, ao):
                n0 = nb * NB
                for oo in range(CT):
                    pp = ps.tile([128, NB], F32, tag="sp", bufs=3)
                    for cc in range(CT):
                        nc.tensor.matmul(
                            pp, w_sb[:, WP, cc, oo * 128:(oo + 1) * 128],
                            ao[:, cc, :],
                            start=(cc == 0), stop=(cc == CT - 1))
                    yf = outp.tile([128, NB], F32, tag="yf")
                    nc.scalar.activation(out=yf, in_=pp, func=AF.Identity,
                                         bias=vec_sb[:, BP, oo:oo + 1])
                    nc.gpsimd.dma_start(
                        out=yf, in_=x[oo * 128:(oo + 1) * 128, n0:n0 + NB],
                        accum_op=OP.add)
                    nc.sync.dma_start(
                        out=y[oo * 128:(oo + 1) * 128, n0:n0 + NB], in_=yf)

            for nb in range(NBLK):
                n0 = nb * NB
                ex = expp.tile([128, MCH, NB], BF16, tag="ex")
                # scores^T chunk j: [m 128, n NB] = k_chunk^T q_block; exp on ACT
                for j in range(MCH):
                    sp = ps.tile([128, NB], F32, tag="sp", bufs=3)
                    for kk in range(CT):
                        nc.tensor.matmul(
                            sp, k_sb[:, kk, j * 128:(j + 1) * 128],
                            q_sb[:, kk, n0:n0 + NB],
                            start=(kk == 0), stop=(kk == CT - 1))
                    nc.scalar.activation(out=ex[:, j, :], in_=sp, func=AF.Exp,
                                         scale=SCL)
                # Z: per-partition chunk sums on DVE, then one ones-matmul to
                # cross partitions; recip + broadcast hide under the AV matmuls
                zr = attp.tile([128, NB], F32, tag="zr")
                nc.vector.reduce_sum(out=zr, in_=ex.rearrange("p j n -> p n j"),
                                     axis=mybir.AxisListType.X)
                zps = ps.tile([1, NB], F32, tag="z")
                nc.tensor.matmul(zps, ones_f32, zr, start=True, stop=True)
                zsb = attp.tile([1, NB], F32, tag="zsb")
                nc.scalar.copy(out=zsb, in_=zps)
                rz = attp.tile([1, NB], F32, tag="rz")
                nc.vector.reciprocal(out=rz, in_=zsb)
                rzb = attp.tile([128, NB], F32, tag="rzb")
                nc.sync.dma_start(
                    out=rzb, in_=_rep_ap(rz, [rz.ap[0], [0, 128], [1, NB]]))
                # previous n-block's proj runs here on PE while this block's
                # softmax-normalize chain (ACT/DVE/DMA) completes
                if proj_pending is not None:
                    proj_block(*proj_pending)
                # U[c, n] = sum_m v[c, m] exp^T[m, n]
                U = ps.tile([128, CT, NB], F32, tag="U")
                for j in range(MCH):
                    for cc in range(CT):
                        nc.tensor.matmul(
                            U[:, cc, :], vT_sb[:, j, cc * 128:(cc + 1) * 128],
                            ex[:, j, :],
                            start=(j == 0), stop=(j == MCH - 1))
                Usb = attp.tile([128, CT, NB], F32, tag="Usb")
                for cc in range(CT):
                    nc.scalar.copy(out=Usb[:, cc, :], in_=U[:, cc, :])
                ao = attp.tile([128, CT, NB], BF16, tag="ao")
                for cc in range(CT):
                    un = attp.tile([128, NB], F32, tag="un")
                    nc.vector.tensor_mul(out=un, in0=Usb[:, cc, :], in1=rzb)
                    nc.vector.tensor_scalar_add(out=ao[:, cc, :], in0=un,
                                                scalar1=vec_sb[:, BV, cc:cc + 1])
                proj_pending = (nb, ao)
            proj_block(*proj_pending)


def _build_program():
    nc = bacc.Bacc()
    t = {}
    t["x_img"] = nc.dram_tensor("x_img", [C, HW], F32, kind="ExternalInput")
    t["xh"] = nc.dram_tensor("xh", [C, HW], BF16, kind="ExternalInput")
    t["wAll"] = nc.dram_tensor("wAll", [len(W_NAMES), C, C], BF16,
                               kind="ExternalInput")
    t["vecs"] = nc.dram_tensor("vecs", [len(V_NAMES), C], F32,
                               kind="ExternalInput")
    t["gmap"] = nc.dram_tensor("gmap", [128, NG], F32, kind="ExternalInput")
    t["gmapT"] = nc.dram_tensor("gmapT", [NG, 128], F32, kind="ExternalInput")
    t["y"] = nc.dram_tensor("y", [C, NH], F32, kind="ExternalOutput")
    with tile.TileContext(nc) as tc:
        _emit(nc, tc, t)
    nc.compile()
    return nc


def _get_program():
    global _NC
    if _NC is None:
        _NC = _build_program()
    return _NC


def _make_in_maps(inputs):
    f32 = np.float32
    bf16 = ml_dtypes.bfloat16
    xs = np.asarray(inputs["x"], f32).reshape(B, C, HW)
    wAll = np.stack([np.asarray(inputs[k], f32).T
                     for k in ("Wq", "Wk", "Wv", "Wp")]).astype(bf16)
    vecs = np.stack([np.asarray(inputs[k], f32)
                     for k in ("bq", "bk", "bv", "bp", "gamma", "beta")])
    gmap = np.zeros((128, NG), f32)
    gmap[np.arange(128), np.arange(128) // GROUP] = 1.0 / GROUP
    gmapT = np.zeros((NG, 128), f32)
    gmapT[np.arange(128) // GROUP, np.arange(128)] = 1.0
    shared = {"wAll": np.ascontiguousarray(wAll),
              "vecs": np.ascontiguousarray(vecs),
              "gmap": gmap, "gmapT": gmapT}
    in_maps = []
    for core in range(N_CORES):
        b, t = core // 2, core % 2
        xi = xs[b]
        if t:
            xi = np.roll(xi, -NH, axis=1)
        xi = np.ascontiguousarray(xi)
        in_maps.append({"x_img": xi, "xh": xi.astype(bf16), **shared})
    return in_maps


def _assemble(results):
    out = np.empty((B, C, HW), np.float32)
    for core in range(N_CORES):
        b, t = core // 2, core % 2
        out[b][:, t * NH:(t + 1) * NH] = results[core]["y"]
    return out.reshape(B, C, HW // 64, 64)


def _run(inputs, **kwargs):
    nc = _get_program()
    in_maps = _make_in_maps(inputs)
    bkr = run_bass_kernel_spmd(nc, in_maps, list(range(N_CORES)), **kwargs)
    return _assemble(bkr.results), bkr


def kernel(**inputs):
    out, _ = _run(inputs)
    return out
